# revision 7
# baseline (speedup 1.0000x reference)
"""Trainium2 Bass kernel for causal GQA self-attention (fused QKV + RoPE).

Problem: B=2, T=2048, C=2048, H=16 q-heads, KV=4 kv-heads, HD=128.
Sharding: 8 cores = (batch b, kv-group k). Each core computes the 4 q-heads
of one kv group for one batch element; outputs are disjoint slices of y.

v4 design (fp16 end-to-end, PE-bound ~151us of matmul):
  - All SBUF data fp16 (1 cycle/row on the PE at any width, half the DMA,
    2x DVE tensor_tensor mode). PSUM stays fp32.
  - qkv^T lives in 24 separate [128,512] tiles (per j-block x t-block) so
    every producer/consumer dependency is tile-exact (no false stalls).
  - Per-tt-first schedule: proj k,v,q0,q1 for tt=0 only, then attention
    passes start immediately; the remaining 20 proj tiles drain inside the
    attention passes at deadline-driven budgets.  This keeps the PE fed
    with x[0]-only work during the ~24us it takes the x stream to land,
    eliminating the input-DMA starvation gaps of the v3 all-proj-first
    schedule.
  - DMA issue is split across both HWDGE engines: x stream on nc.scalar
    (Act idle during startup), weights/consts/outputs on nc.sync.  Exp
    activation table prewarmed at t=0; all PSUM->SBUF copies on DVE so
    Act only ever runs Exp (no table-set reloads).
  - Attention processes head pairs with two-step score lookahead:
    PSUM = 2x1 banks proj (shared with V-transpose) + 2x2 banks scores
    + 2 banks y = 8.
  - Row sums of exp accumulate on DVE (fp16 tensor_tensor, 2x mode);
    the 128-partition reduce + divide happen on host.
  - V transposed to s-major by PE matmul against identity (v^T as
    stationary); RoPE via SBUF->SBUF partition-swap DMA + 3 tensor_tensor.
  - Final pass drains y and acc in column chunks as PV/scores finalize
    them, shortening the serial tail after the last matmul.
Output per core: unnormalized y^T [128, 4, 2048] fp16 + exp-sum tiles
[8, 128, 1024] fp16; host reduces, divides, transposes, concatenates.
"""

import math

import numpy as np

import concourse.bass as bass
import concourse.mybir as mybir
import concourse.tile as tile
from concourse import bacc
from concourse.bass_utils import run_bass_kernel_spmd

B, T, C = 2, 2048, 2048
H, KV, HD = 16, 4, 128
NREP = H // KV  # q heads per core
P = 128
NCORES = 8
CC = C // P  # 16 contraction chunks
TT = 4  # t-blocks of 512
TB = T // TT  # 512
SCALE = 1.0 / math.sqrt(HD)

f16 = mybir.dt.float16
f32 = mybir.dt.float32

TRACE = False  # set True (with ntff shim installed) to get exec_time_ns

_cache = {}


def _build():
    if "nc" in _cache:
        return _cache["nc"]

    nc = bacc.Bacc("TRN2", target_bir_lowering=False, debug=False,
                   num_devices=NCORES)

    # DRAM inputs (pre-laid-out on host for contiguous per-partition DMA)
    xT_d = nc.dram_tensor("xT", [TT, P, 4, 4, TB], f16, kind="ExternalInput").ap()
    wT_d = nc.dram_tensor("wT", [6, P, 4, 4, P], f16, kind="ExternalInput").ap()
    cc_d = nc.dram_tensor("CC", [P, T], f16, kind="ExternalInput").ap()
    ss_d = nc.dram_tensor("SS2", [P, T], f16, kind="ExternalInput").ap()
    tri_d = nc.dram_tensor("tri", [P, P], f16, kind="ExternalInput").ap()
    id_d = nc.dram_tensor("ident", [P, P], f16, kind="ExternalInput").ap()
    # outputs: y^T d-major [d, head, t], exp-sums per (tb, pass)
    yT_d = nc.dram_tensor("yT", [P, NREP, T], f16, kind="ExternalOutput").ap()
    acc_d = nc.dram_tensor("acc", [TT * 2, P, 2 * TB], f16,
                           kind="ExternalOutput").ap()

    mult = mybir.AluOpType.mult
    add = mybir.AluOpType.add

    with tile.TileContext(nc) as tc:
        with (
            tc.tile_pool(name="big", bufs=1) as big_pool,
            tc.tile_pool(name="swp", bufs=2) as swp_pool,
            tc.tile_pool(name="ropetmp", bufs=2) as rt_pool,
            tc.tile_pool(name="expt", bufs=4) as exp_pool,
            tc.tile_pool(name="accp", bufs=2) as acc_pool,
            tc.tile_pool(name="yout", bufs=2) as y_pool,
            tc.tile_pool(name="warm", bufs=1) as warm_pool,
            tc.tile_pool(name="pp", bufs=2, space="PSUM") as pp_pool,
            tc.tile_pool(name="sp", bufs=2, space="PSUM") as sp_pool,
            tc.tile_pool(name="yp", bufs=1, space="PSUM") as yp_pool,
        ):
            # ---- exp table prewarm: Act loads the Exp table set at t=0,
            # before the first real exp ~20us in (one-time ~2.7us cost).
            warm = warm_pool.tile([P, 8], f16, tag="warm")
            warm2 = warm_pool.tile([P, 8], f16, tag="warm2")
            nc.vector.memset(warm[:], 0.0)
            nc.scalar.activation(warm2[:], warm[:],
                                 mybir.ActivationFunctionType.Exp, scale=1.0)

            # ---- resident tensors ----
            w_sb = big_pool.tile([P, 6, 4, 4, P], f16, tag="w")
            x_sb = big_pool.tile([P, TT, 4, 4, TB], f16, tag="x")
            # qkv^T as separate tiles per (j-block, t-block): exact deps
            qkv = [[big_pool.tile([P, TB], f16, tag=f"qkv{j}_{t}",
                                  name=f"qkv{j}_{t}")
                    for t in range(TT)] for j in range(6)]
            v_sb = big_pool.tile([P, CC, P], f16, tag="v")
            ccs = big_pool.tile([P, T], f16, tag="cc")
            ss2 = big_pool.tile([P, T], f16, tag="ss")
            tri = big_pool.tile([P, P], f16, tag="tri")
            ident = big_pool.tile([P, P], f16, tag="ident")

            # ---- startup DMA, split across the two HWDGE engines and
            # ordered by consumption: k/v weights + x[0] first (critical
            # path to the first matmuls), q2/q3 weights last.
            # sync: weights + constants.  scalar: the x stream.
            nc.sync.dma_start(w_sb[:, 4, 0], wT_d[4, :, 0])
            nc.scalar.dma_start(x_sb[:, 0, 0, 0, :], xT_d[0, :, 0, 0, :])
            nc.sync.dma_start(w_sb[:, 4, 1:4], wT_d[4, :, 1:4])
            nc.scalar.dma_start(x_sb[:, 0, 0, 1:4, :], xT_d[0, :, 0, 1:4, :])
            nc.sync.dma_start(w_sb[:, 5], wT_d[5])
            nc.scalar.dma_start(x_sb[:, 0, 1, :, :], xT_d[0, :, 1])
            nc.sync.dma_start(w_sb[:, 0], wT_d[0])
            nc.scalar.dma_start(x_sb[:, 0, 2, :, :], xT_d[0, :, 2])
            nc.sync.dma_start(w_sb[:, 1], wT_d[1])
            nc.scalar.dma_start(x_sb[:, 0, 3, :, :], xT_d[0, :, 3])
            nc.sync.dma_start(ccs[:], cc_d[:])
            nc.sync.dma_start(ss2[:], ss_d[:])
            nc.sync.dma_start(tri[:], tri_d[:])
            nc.sync.dma_start(ident[:], id_d[:])
            for tt in range(1, TT):
                nc.scalar.dma_start(x_sb[:, tt], xT_d[tt])
            nc.sync.dma_start(w_sb[:, 2], wT_d[2])
            nc.sync.dma_start(w_sb[:, 3], wT_d[3])

            def proj_one(j, tt):
                """Project j-block j for t-block tt into qkv[j][tt]."""
                ps = pp_pool.tile([P, TB], f32, tag="pp", name="ps")
                for cq in range(4):
                    for ci in range(4):
                        cc = cq * 4 + ci
                        nc.tensor.matmul(
                            ps[:],
                            w_sb[:, j, cq, ci, :],
                            x_sb[:, tt, cq, ci, :],
                            start=(cc == 0),
                            stop=(cc == CC - 1),
                        )
                nc.vector.tensor_copy(qkv[j][tt][:], ps[:])

            def rope(j, tt):
                """In-place rotate-half RoPE on qkv[j][tt]."""
                tsl = slice(tt * TB, (tt + 1) * TB)
                q = qkv[j][tt]
                swp = swp_pool.tile([P, TB], f16, tag="swp", name="swp")
                nc.sync.dma_start(swp[0:64, :], q[64:128, :])
                nc.sync.dma_start(swp[64:128, :], q[0:64, :])
                ta = rt_pool.tile([P, TB], f16, tag="ta", name="ta")
                tb_ = rt_pool.tile([P, TB], f16, tag="tb", name="tb")
                nc.vector.tensor_tensor(ta[:], q[:], ccs[:, tsl], mult)
                nc.vector.tensor_tensor(tb_[:], swp[:], ss2[:, tsl], mult)
                nc.vector.tensor_tensor(q[:], ta[:], tb_[:], add)

            def vtrans(tt):
                """v^T [d, s] chunks -> v_sb [s, chunk, d] via PE matmul
                with v^T stationary and identity moving."""
                ps = pp_pool.tile([P, TB], f32, tag="pp", name="vtr")
                for i in range(4):
                    nc.tensor.matmul(
                        ps[:, i * P:(i + 1) * P],
                        qkv[5][tt][:, i * P:(i + 1) * P],
                        ident[:],
                        start=True, stop=True,
                    )
                nc.vector.tensor_copy(v_sb[:, 4 * tt:4 * tt + 4, :], ps[:])

            # ---- attention for one (tb, head-pair) with score lookahead ----
            def att_pass(tb, h0, interleave=None, chunked_tail=False):
                nsc = 4 * (tb + 1)
                depth = 2
                yp = yp_pool.tile([P, 2, TB], f32, tag="yp", name="yp")
                acc = acc_pool.tile([P, 2, TB], f16, tag="acc", name="acc")
                exts = [None] * nsc

                def col0(sc):
                    r = sc - 4 * tb
                    return r * P if r >= 0 else 0

                def scores(sc):
                    c0 = col0(sc)
                    sp = sp_pool.tile([P, 2, TB], f32, tag="sp", name="sp")
                    for k in range(2):
                        nc.tensor.matmul(
                            sp[:, k, c0:],
                            qkv[4][sc // 4][:, (sc % 4) * P:(sc % 4 + 1) * P],
                            qkv[h0 + k][tb][:, c0:],
                            start=True, stop=True,
                        )
                    ex = exp_pool.tile([P, 2, TB], f16, tag="ex", name="ex")
                    nc.scalar.activation(
                        ex[:, :, c0:], sp[:, :, c0:],
                        mybir.ActivationFunctionType.Exp, scale=SCALE)
                    if sc - 4 * tb >= 0:
                        for k in range(2):
                            nc.vector.tensor_tensor(
                                ex[:, k, c0:c0 + P], ex[:, k, c0:c0 + P],
                                tri[:], mult)
                    if sc == 0:
                        nc.vector.tensor_copy(acc[:], ex[:])
                    else:
                        nc.vector.tensor_tensor(
                            acc[:, :, c0:], ex[:, :, c0:], acc[:, :, c0:], add)
                    exts[sc] = ex
                    # chunked acc drain: after the diag step r, acc columns
                    # [rP,(r+1)P) are final for both heads.  Drain per-head
                    # halves as they complete to shorten the final tail.
                    if chunked_tail and sc - 4 * tb == 1:
                        for k in range(2):
                            nc.sync.dma_start(
                                acc_d[tb * 2 + h0 // 2, :,
                                      k * TB:k * TB + 2 * P],
                                acc[:, k, 0:2 * P])
                    if chunked_tail and sc - 4 * tb == 3:
                        for k in range(2):
                            nc.sync.dma_start(
                                acc_d[tb * 2 + h0 // 2, :,
                                      k * TB + 2 * P:(k + 1) * TB],
                                acc[:, k, 2 * P:])

                def pv(sc):
                    c0 = col0(sc)
                    for k in range(2):
                        nc.tensor.matmul(
                            yp[:, k, c0:],
                            v_sb[:, sc, :],
                            exts[sc][:, k, c0:],
                            start=(sc == 0), stop=(sc == nsc - 1),
                        )

                for sc in range(nsc):
                    scores(sc)
                    if interleave is not None:
                        interleave(sc)
                    if sc >= depth:
                        pv(sc - depth)
                if not chunked_tail:
                    # acc is complete after the last scores step: drain now
                    nc.sync.dma_start(acc_d[tb * 2 + h0 // 2], acc[:])
                    for sc in range(nsc - depth, nsc):
                        pv(sc)
                    ysb = y_pool.tile([P, 2, TB], f16, tag="ysb", name="ysb")
                    # per-head copy + DMA so the final drain starts earlier
                    for k in range(2):
                        nc.vector.tensor_copy(ysb[:, k, :], yp[:, k, :])
                        nc.sync.dma_start(
                            yT_d[:, h0 + k, tb * TB:(tb + 1) * TB],
                            ysb[:, k, :])
                else:
                    # final pass: y columns [0,256) are final after
                    # pv(nsc-3), [256,384) after pv(nsc-2), [384,512)
                    # after pv(nsc-1).  Copy+DMA each chunk as it lands so
                    # cast/DMA overlap the remaining PV matmuls.
                    ysb = y_pool.tile([P, 2, TB], f16, tag="ysb", name="ysb")
                    chunks = [(0, 2 * P), (2 * P, 3 * P), (3 * P, TB)]
                    for i, sc in enumerate(range(nsc - depth, nsc)):
                        pv(sc)
                        # chunk i of y is final after pv(nsc-3+i); the
                        # first chunk lands right after the first tail pv.
                        flo, fhi = chunks[i]
                        for k in range(2):
                            nc.vector.tensor_copy(
                                ysb[:, k, flo:fhi], yp[:, k, flo:fhi])
                            nc.sync.dma_start(
                                yT_d[:, h0 + k,
                                     tb * TB + flo:tb * TB + fhi],
                                ysb[:, k, flo:fhi])
                    flo, fhi = chunks[2]
                    for k in range(2):
                        nc.vector.tensor_copy(
                            ysb[:, k, flo:fhi], yp[:, k, flo:fhi])
                        nc.sync.dma_start(
                            yT_d[:, h0 + k, tb * TB + flo:tb * TB + fhi],
                            ysb[:, k, flo:fhi])

            # ---- schedule ----
            # initial block: k, v, q0, q1 for tt=0 only (x[0]-dependent)
            proj_one(4, 0)
            rope(4, 0)
            proj_one(5, 0)
            vtrans(0)
            proj_one(0, 0)
            rope(0, 0)
            proj_one(1, 0)
            rope(1, 0)

            # remaining 20 proj tiles drain inside the attention passes.
            # Order = deadline order: tiles needed by pass p+1 drain during
            # pass p.  One cq-quarter (4 matmuls) or one rope/vtrans/copy
            # block per yield.
            def gen_drain():
                # (j, tt) in deadline order: q2,q3 of tt before k,v,q0,q1
                # of tt+1 (pass (tt,2) precedes pass (tt+1,0))
                order = [(2, 0), (3, 0)]
                for tt in range(1, TT):
                    order += [(4, tt), (5, tt), (0, tt), (1, tt),
                              (2, tt), (3, tt)]
                for j, tt in order:
                    ps = pp_pool.tile([P, TB], f32, tag="pp", name="ps")
                    for cq in range(4):
                        for ci in range(4):
                            cc = cq * 4 + ci
                            nc.tensor.matmul(
                                ps[:],
                                w_sb[:, j, cq, ci, :],
                                x_sb[:, tt, cq, ci, :],
                                start=(cc == 0),
                                stop=(cc == CC - 1),
                            )
                        yield
                    nc.vector.tensor_copy(qkv[j][tt][:], ps[:])
                    if j == 5:
                        vtrans(tt)
                    else:
                        rope(j, tt)
                    yield

            it = gen_drain()

            # Budgets: yields needed before each subsequent pass starts.
            # tiles: q=5 yields, k=5, v=6 (copy+vtrans counts 1... both
            # are a single trailing yield) -> every tile is 5 yields.
            # p0 (0,0): drain q2_0,q3_0 (10) for p1
            # p1 (0,2): drain k,v,q0,q1 @tt=1 (20) for p2
            # p2 (1,0): drain q2_1,q3_1 (10) for p3
            # p3 (1,2): drain kvq01 @tt=2 (20) for p4
            # p4 (2,0): drain q2_2,q3_2 (10) for p5
            # p5 (2,2): drain kvq01 @tt=3 (20) for p6
            # p6 (3,0): drain q2_3,q3_3 (10) for p7
            # p7 (3,2): bare
            passes = [(0, 0, 10), (0, 2, 20), (1, 0, 10), (1, 2, 20),
                      (2, 0, 10), (2, 2, 20), (3, 0, 10), (3, 2, 0)]

            def make_drain(budget, nsc):
                done = [0]

                def drain(sc):
                    target = budget * (sc + 1) // nsc
                    while done[0] < target:
                        try:
                            next(it)
                        except StopIteration:
                            return
                        done[0] += 1
                return drain

            for tb, h0, budget in passes:
                att_pass(tb, h0, interleave=make_drain(budget, 4 * (tb + 1)),
                         chunked_tail=(tb == 3 and h0 == 2))
            for _ in it:  # finish any leftover proj work
                pass

    nc.compile()
    _cache["nc"] = nc
    return nc


def _host_prep(x, w_qkv, freqs_cos, freqs_sin):
    """Build per-core input maps (numpy, cheap)."""
    x = np.asarray(x, dtype=np.float32)
    w_qkv = np.asarray(w_qkv, dtype=np.float32)
    freqs_cos = np.asarray(freqs_cos, dtype=np.float32)
    freqs_sin = np.asarray(freqs_sin, dtype=np.float32)

    perm = np.concatenate([np.arange(0, HD, 2), np.arange(1, HD, 2)])

    # x^T per batch in [tt, p, cq, ci, tb] layout (16KB contiguous per
    # partition per t-block: whole-tt DMAs use one descriptor per row)
    xTs = []
    for b in range(B):
        xt = x[b].T.reshape(4, 4, P, TT, TB).transpose(3, 2, 0, 1, 4)
        xTs.append(np.ascontiguousarray(xt.astype(np.float16)))

    cosT = freqs_cos.T  # [64, T]
    sinT = freqs_sin.T
    CCh = np.ascontiguousarray(
        np.concatenate([cosT, cosT], axis=0).astype(np.float16))
    SS2 = np.ascontiguousarray(
        np.concatenate([-sinT, sinT], axis=0).astype(np.float16))
    tri = np.triu(np.ones((P, P), dtype=np.float16))
    ident = np.eye(P, dtype=np.float16)

    in_maps = []
    for core in range(NCORES):
        b, kv = divmod(core, KV)
        blocks = []
        for r in range(NREP):
            hrow = (kv * NREP + r) * HD
            blocks.append(w_qkv[hrow:hrow + HD][perm])
        blocks.append(w_qkv[H * HD + kv * HD:H * HD + (kv + 1) * HD][perm])
        blocks.append(
            w_qkv[(H + KV) * HD + kv * HD:(H + KV) * HD + (kv + 1) * HD]
        )
        w_shard = np.concatenate(blocks, axis=0)  # [768, C]
        # [j, p, cq, ci, 128]: c = (cq*4+ci)*128+p, col j*128+d
        wT = w_shard.T.reshape(4, 4, P, 6, P).transpose(3, 2, 0, 1, 4)
        wT = np.ascontiguousarray(wT.astype(np.float16))
        in_maps.append({
            "xT": xTs[b],
            "wT": wT,
            "CC": CCh,
            "SS2": SS2,
            "tri": tri,
            "ident": ident,
        })
    return in_maps


def kernel(x, w_qkv, freqs_cos, freqs_sin):
    nc = _build()
    in_maps = _host_prep(x, w_qkv, freqs_cos, freqs_sin)
    res = run_bass_kernel_spmd(nc, in_maps, list(range(NCORES)), trace=TRACE)
    _cache["last_res"] = res

    y = np.empty((B, T, C), dtype=np.float32)
    for core in range(NCORES):
        b, kv = divmod(core, KV)
        yT = res.results[core]["yT"].astype(np.float32)  # [P, NREP, T]
        accs = res.results[core]["acc"].astype(np.float32)  # [8, P, 2*TB]
        acc = accs.reshape(TT, 2, P, 2, TB)  # [tb, pass, lane, hh, t]
        den = acc.sum(axis=2)  # [tb, pass, hh, t]
        den = den.transpose(1, 2, 0, 3).reshape(NREP, T)  # [h, t]
        y_norm = yT / den[None, :, :]  # [d, h, t]
        y[b, :, kv * NREP * HD:(kv + 1) * NREP * HD] = (
            y_norm.transpose(2, 1, 0).reshape(T, NREP * HD)
        )
    return y
